# revision 22
# baseline (speedup 1.0000x reference)
"""Trainium2 Bass kernel for a pre-norm transformer block (E=512, H=2048, NH=8, N=4096).

Sequence-parallel over 8 NeuronCores: each core computes the full K/V
projection but only its own 512-token slice of queries, attention output, MLP
and residuals. Inputs are rotated per core so chunk 0 is always the core's own
slice (softmax/PV are key-order invariant), which lets the SPMD program
compute Q during chunk 0 with no core-dependent control flow.

Pipeline: the 4096 tokens stream in 8 chunks of 512. Chunk ch does LN1 +
QKV projection (fp8 DoubleRow, LN gamma/beta folded into weights host-side,
stats via a ones[128,2,128] DoubleRow matmul whose [128,CW] output is the
per-token sum replicated across partitions), and simultaneously runs
attention (scores -> exp -> PV) against chunk ch-1's K/V, which live in a
2-slot ring. The ACT engine is the bottleneck (exp is ~1 elem/cycle/lane and
only runs there), so everything else is arranged to hide under it: scores are
bf16 row-tiled (2 heads concurrently on 64-row halves of the PE), PV is fp8
DoubleRow over kt pairs with the softmax denominator riding as a 65th column
of V, and per-chunk PV partials spill from PSUM into an SBUF accumulator.
exp carries a constant bias -EXPB (softmax-invariant) to keep values in fp8
range. rstd = exp(-0.5*ln(var+eps)) on ACT: ln/exp share one table set, so
the whole kernel needs a single ACT table load.

Denominator: 1/den via ACT ln/exp on the [1, 4096] den row, broadcast to 64
partitions with a K=1 matmul against a ones[1,64] stationary, then one DVE
multiply; per-head results DMA into the feature-major UTs frame. The MLP
(proj -> LN2 -> fc1 -> fc2 -> fc3) is bf16 (accuracy) and PE-bound; it runs
after the last chunk since attention for every query needs all keys.
"""
import sys

sys.path.insert(0, "/opt/trn_rl_repo")
sys.path.insert(0, "/opt/pypackages")

import numpy as np

E, H, NH, HD = 512, 2048, 8, 64
T, NCORES = 4096, 8
TC = T // NCORES          # tokens per core
P = 128
ET = E // P               # 4  feature tiles of E
HT = H // P               # 16 feature tiles of H
EPS = 1e-5
WS = 16.0                 # fp8 weight pre-scale
CW = 512                  # chunk width (tokens)
NCH = T // CW
KB = CW // P              # 4 key blocks per chunk
PADV = 80                 # V pair stride (16-aligned) for DoubleRow stationary
EXPB = 2.0

_BUILT = None


def _build():
    import concourse.bacc as bacc
    import concourse.mybir as mybir
    import concourse.tile as tile

    nc = bacc.Bacc("TRN2", target_bir_lowering=False, debug=False, num_devices=NCORES)
    dt = mybir.dt
    F32, F8, BF = dt.float32, dt.float8e4, dt.bfloat16

    d = {}
    d["d_xT8"] = nc.dram_tensor("xT8", [E, T], F8, kind="ExternalInput").ap()
    d["d_xsT"] = nc.dram_tensor("xsT", [E, TC], F32, kind="ExternalInput").ap()
    d["d_wqkvT8"] = nc.dram_tensor("wqkvT8", [E, 3 * E], F8, kind="ExternalInput").ap()
    d["d_bqkv"] = nc.dram_tensor("bqkv", [3 * E], F32, kind="ExternalInput").ap()
    d["d_wprojT"] = nc.dram_tensor("wprojT", [E, E], BF, kind="ExternalInput").ap()
    d["d_bproj"] = nc.dram_tensor("bproj", [E], F32, kind="ExternalInput").ap()
    d["d_wfc1T"] = nc.dram_tensor("wfc1T", [E, H], BF, kind="ExternalInput").ap()
    d["d_bfc1"] = nc.dram_tensor("bfc1", [H], F32, kind="ExternalInput").ap()
    d["d_wfc2T"] = nc.dram_tensor("wfc2T", [H, H], BF, kind="ExternalInput").ap()
    d["d_bfc2"] = nc.dram_tensor("bfc2", [H], F32, kind="ExternalInput").ap()
    d["d_wfc3T"] = nc.dram_tensor("wfc3T", [H, E], BF, kind="ExternalInput").ap()
    d["d_bfc3"] = nc.dram_tensor("bfc3", [E], F32, kind="ExternalInput").ap()
    d["d_outT"] = nc.dram_tensor("outT", [E, TC], F32, kind="ExternalOutput").ap()

    with tile.TileContext(nc) as tc:
        _emit(nc, tc, tile, mybir, d)

    nc.compile()
    return nc


def _emit(nc, tc, tile, mybir, d):
    dt = mybir.dt
    AF = mybir.ActivationFunctionType
    OP = mybir.AluOpType
    PM = mybir.MatmulPerfMode
    F32, F8, BF = dt.float32, dt.float8e4, dt.bfloat16
    DR = PM.DoubleRow
    scale = float(HD) ** -0.5

    def pool(**kw):
        p = tc.tile_pool(**kw)
        return p.__enter__(), p

    def close(*ps):
        for p in ps:
            p.__exit__(None, None, None)

    # ---- long-lived pools ----
    consts, _c0 = pool(name="consts", bufs=1, side="left")
    lnp, _c1 = pool(name="lnp", bufs=2, side="left")

    # ---- constants ----
    ones_w = consts.tile([P, 1], BF)
    nc.vector.memset(ones_w[:], 1.0)
    ones8p = consts.tile([P, 2, P], F8)        # DoubleRow stats stationary
    nc.vector.memset(ones8p[:], 1.0)
    ones_bf = consts.tile([P, P], BF)          # bf16 stats stationary (LN2)
    nc.vector.memset(ones_bf[:], 1.0)
    onesPr = consts.tile([P, HD], F32)         # K=1 broadcast stationary (row 64)
    nc.vector.memset(onesPr[:], 1.0)
    eps_p = consts.tile([P, 1], F32)
    nc.vector.memset(eps_p[:], EPS)
    nexpb_p = consts.tile([P, 1], F32)
    nc.vector.memset(nexpb_p[:], -EXPB)

    def ld_vec(dram, n, name):  # [n] f32 -> [P, n//P] per-partition layout
        t = consts.tile([P, n // P], F32, name=name)
        nc.sync.dma_start(t[:], dram.rearrange("(m p) -> p m", p=P))
        return t

    bq_sb = ld_vec(d["d_bqkv"][0:E], E, "bq_sb")
    bk_sb = ld_vec(d["d_bqkv"][E:2 * E], E, "bk_sb")
    bv_sb = ld_vec(d["d_bqkv"][2 * E:3 * E], E, "bv_sb")
    bproj_sb = ld_vec(d["d_bproj"], E, "bproj_sb")
    bfc1_sb = ld_vec(d["d_bfc1"], H, "bfc1_sb")
    bfc2_sb = ld_vec(d["d_bfc2"], H, "bfc2_sb")
    bfc3_sb = ld_vec(d["d_bfc3"], E, "bfc3_sb")

    def ln_chain(mu_ps, sq_ps, w):
        """mu_ps/sq_ps: [P, w] PSUM, per-token sum(x)/sum(x^2) replicated over
        partitions. Returns (mu_b, rs_b) bf16 [P, w]."""
        mu_b = lnp.tile([P, w], BF, tag="mu", name="mu_b")
        nc.vector.tensor_scalar_mul(mu_b[:], mu_ps[:], 1.0 / E)
        var = lnp.tile([P, w], F32, tag="va", name="var")
        nc.gpsimd.tensor_mul(var[:], mu_b[:], mu_b[:])
        nc.vector.scalar_tensor_tensor(var[:], sq_ps[:], 1.0 / E, var[:],
                                       op0=OP.mult, op1=OP.subtract)
        lnv = lnp.tile([P, w], F32, tag="lv", name="lnv")
        nc.scalar.activation(lnv[:], var[:], AF.Ln, bias=eps_p[:])
        rs_b = lnp.tile([P, w], BF, tag="rs", name="rs_b")
        nc.scalar.activation(rs_b[:], lnv[:], AF.Exp, scale=-0.5)
        return mu_b, rs_b

    # ---- persistent tensors ----
    persistA, h_persistA = pool(name="persistA", bufs=1, side="left")
    QTs = persistA.tile([P, ET, TC], BF)
    UACC = persistA.tile([P, NH, TC], F32)      # PV accumulator (rows 0..64 live)
    UTs = persistA.tile([P, ET, TC], BF)        # normalized attention out
    xs_sb = persistA.tile([P, ET, TC], F32)
    nc.sync.dma_start(xs_sb[:], d["d_xsT"].rearrange("(m p) t -> p m t", p=P))

    # MLP weights (DMA streams during the attention window)
    wpp, h_wpp = pool(name="wproj", bufs=1, side="left")
    wproj = wpp.tile([P, ET, E], BF)
    w1p, h_w1p = pool(name="wfc1", bufs=1, side="left")
    wfc1 = w1p.tile([P, ET, H], BF)
    w3p, h_w3p = pool(name="wfc3", bufs=1, side="left")
    wfc3 = w3p.tile([P, HT, E], BF)
    w2p, h_w2p = pool(name="wfc2c", bufs=1, side="left")
    wcs = []

    # ---- rolling K/V ring + weights + chunk pools ----
    wq8p, h_wq8p = pool(name="wq8", bufs=1, side="right")
    wqkv8 = wq8p.tile([P, ET, 3 * E], F8)
    kvp, h_kvp = pool(name="kvring", bufs=1, side="right")
    KT2 = kvp.tile([P, 2, ET, CW], BF)          # K^T ring, feature-major
    V65 = kvp.tile([P, 2, KB // 2, NH, 2, PADV], F8)  # V ring, DR pairs + ones col
    nc.vector.memset(V65[:, :, :, :, :, HD:HD + 1], 1.0)
    xcp, h_xcp = pool(name="xc", bufs=3, side="right")
    xnp, h_xnp = pool(name="xn", bufs=2, side="right")
    sqp, h_sqp = pool(name="sq", bufs=2, side="right")
    ptp, h_ptp = pool(name="ptile", bufs=4, side="right")
    stp, h_stp = pool(name="stage", bufs=4, side="right")

    # ---- PSUM: mm ring (2) + scores (2x2) + pv (2) = 8 banks ----
    ps_mm, h_ps_mm = pool(name="ps_mm", bufs=2, space="PSUM")
    ps_sc, h_ps_sc = pool(name="ps_sc", bufs=2, space="PSUM")
    ps_pv, h_ps_pv = pool(name="ps_pv", bufs=2, space="PSUM")

    def warmup(n, rhs, wp):
        wps = wp.tile([1, rhs.shape[-1]], F32, tag="mm", name="wps")
        for i in range(n):
            nc.tensor.matmul(wps[:], ones_w[:], rhs,
                             start=(i == 0), stop=(i == n - 1),
                             skip_group_check=True)

    def ln_stats(xc8, w):
        xsq = sqp.tile([P, ET, w], F8, tag="xsq", name="xsq")
        for e in range(ET):
            eng = nc.gpsimd if e < 2 else nc.vector
            eng.tensor_mul(xsq[:, e, :], xc8[:, e, :], xc8[:, e, :])
        mu_ps = ps_mm.tile([P, w], F32, tag="mm", name="mu_ps")
        for h in range(2):
            nc.tensor.matmul(mu_ps[:], ones8p[:], xc8[:, 2 * h:2 * h + 2, :],
                             start=(h == 0), stop=(h == 1), perf_mode=DR)
        sq_ps = ps_mm.tile([P, w], F32, tag="mm", name="sq_ps")
        for h in range(2):
            nc.tensor.matmul(sq_ps[:], ones8p[:], xsq[:, 2 * h:2 * h + 2, :],
                             start=(h == 0), stop=(h == 1), perf_mode=DR)
        return ln_chain(mu_ps, sq_ps, w)

    def ln_apply8(xn8, xc8, mu_b, rs_b, w):
        for e in range(ET):
            tmp = lnp.tile([P, w], BF, tag=f"ap{e & 1}", name="tmp")
            eng = nc.gpsimd if e < 2 else nc.vector
            eng.tensor_sub(tmp[:], xc8[:, e, :], mu_b[:])
            nc.vector.tensor_mul(xn8[:, e, :], tmp[:], rs_b[:])

    def q_project(xn8):
        for m in range(ET):
            qps = ps_mm.tile([P, TC], F32, tag="mm", name="qps")
            for h in range(2):
                nc.tensor.matmul(
                    qps[:], wqkv8[:, 2 * h:2 * h + 2, m * P:(m + 1) * P],
                    xn8[:, 2 * h:2 * h + 2, :],
                    start=(h == 0), stop=(h == 1), perf_mode=DR)
            nc.vector.tensor_scalar(QTs[:, m, :], qps[:], 1.0 / WS,
                                    bq_sb[:, m:m + 1], op0=OP.mult, op1=OP.add)

    def kv_project(xn8, ch):
        r = ch % 2
        for m in range(ET):
            kps = ps_mm.tile([P, CW], F32, tag="mm", name="kps")
            for h in range(2):
                nc.tensor.matmul(
                    kps[:], wqkv8[:, 2 * h:2 * h + 2, E + m * P:E + (m + 1) * P],
                    xn8[:, 2 * h:2 * h + 2, :],
                    start=(h == 0), stop=(h == 1), perf_mode=DR)
            nc.vector.tensor_scalar(KT2[:, r, m, :], kps[:], 1.0 / WS,
                                    bk_sb[:, m:m + 1], op0=OP.mult, op1=OP.add)
        for t4 in range(KB):
            vps = ps_mm.tile([P, E], F32, tag="mm", name="vps")
            for h in range(2):
                nc.tensor.matmul(
                    vps[:], xn8[:, 2 * h:2 * h + 2, t4 * P:(t4 + 1) * P],
                    wqkv8[:, 2 * h:2 * h + 2, 2 * E:3 * E],
                    start=(h == 0), stop=(h == 1), perf_mode=DR)
            nc.vector.tensor_scalar_mul(
                V65[:, r, t4 // 2, :, t4 % 2, 0:HD],
                vps[:].rearrange("p (h d) -> p h d", h=NH), 1.0 / WS)

    def normalize_pair(hp):
        """1/den via in-place ACT ln/exp on the den row (partition 64);
        broadcast to 64 partitions with a K=1 matmul; scale numerators; DMA
        into the feature-major UTs frame; add the V bias there."""
        h0 = 2 * hp
        den = UACC[HD:HD + 1, h0:h0 + 2, :]
        nc.scalar.activation(den, den, AF.Ln)
        nc.scalar.activation(den, den, AF.Exp, scale=-1.0)
        for j in range(2):
            h = h0 + j
            lo = (h % 2) * HD
            m = h // 2
            ib = ps_pv.tile([HD, TC], F32, tag="pv", name="ib")
            nc.tensor.matmul(ib[:], onesPr[HD:HD + 1, :], UACC[HD:HD + 1, h, :],
                             skip_group_check=True)
            stg = stp.tile([HD, TC], BF, tag="stg", name="stg")
            nc.vector.tensor_mul(stg[:], UACC[0:HD, h, :], ib[:])
            nc.sync.dma_start(UTs[lo:lo + HD, m, :], stg[:])
            nc.vector.tensor_scalar_add(UTs[lo:lo + HD, m, :],
                                        UTs[lo:lo + HD, m, :],
                                        scalar1=bv_sb[lo:lo + HD, m:m + 1])

    def attention(pch):
        """Scores+exp+PV for chunk pch's keys (ring slot pch%2), all heads."""
        r = pch % 2
        for hp in range(ET):
            heads = [2 * hp, 2 * hp + 1]
            pvt = [None, None]
            for ktp in range(KB // 2):
                scs = []
                for j, h in enumerate(heads):
                    scs.append(ps_sc.tile([P, 2, TC], F32, tag="sc", name="sc"))
                for kt2 in range(2):
                    kb = 2 * ktp + kt2
                    for j, h in enumerate(heads):
                        lo = (h % 2) * HD
                        m = h // 2
                        nc.tensor.matmul(scs[j][:, kt2, :],
                                         KT2[lo:lo + HD, r, m, kb * P:(kb + 1) * P],
                                         QTs[lo:lo + HD, m, :],
                                         skip_group_check=True)
                pts = []
                for j, h in enumerate(heads):
                    pt = ptp.tile([P, 2, TC], F8, tag="pt", name="pt")
                    nc.scalar.activation(pt[:], scs[j][:], AF.Exp,
                                         scale=scale, bias=nexpb_p[:])
                    pts.append(pt)
                for j, h in enumerate(heads):
                    if ktp == 0:
                        pvt[j] = ps_pv.tile([HD + 1, TC], F32, tag="pv", name="pv")
                    nc.tensor.matmul(pvt[j][:], V65[:, r, ktp, h, :, 0:HD + 1],
                                     pts[j][:],
                                     start=(ktp == 0), stop=(ktp == KB // 2 - 1),
                                     perf_mode=DR, skip_group_check=True)
            for j, h in enumerate(heads):
                if pch == 0:
                    nc.vector.tensor_copy(UACC[0:HD + 1, h, :], pvt[j][:])
                else:
                    nc.vector.tensor_add(UACC[0:HD + 1, h, :],
                                         UACC[0:HD + 1, h, :], pvt[j][:])
            if pch == NCH - 1:
                normalize_pair(hp)

    # ====== main pipeline ======
    xns = {}
    for ch in range(NCH):
        xc8 = xcp.tile([P, ET, CW], F8, tag="xc", name="xc")
        nc.sync.dma_start(
            xc8[:],
            d["d_xT8"][:, ch * CW:(ch + 1) * CW].rearrange("(m p) t -> p m t", p=P))
        if ch == 0:
            warmup(10, ones_bf[:, 0:P], ps_mm)
            nc.sync.dma_start(
                wqkv8[:], d["d_wqkvT8"].rearrange("(m p) o -> p m o", p=P))
        elif ch == 1:
            nc.sync.dma_start(wproj[:],
                              d["d_wprojT"].rearrange("(m p) o -> p m o", p=P))
            nc.sync.dma_start(wfc1[:],
                              d["d_wfc1T"].rearrange("(m p) o -> p m o", p=P))
            nc.sync.dma_start(wfc3[:],
                              d["d_wfc3T"].rearrange("(m p) o -> p m o", p=P))
        if ch >= 1 and ch <= 4:
            for e in range(4 * (ch - 1), 4 * ch):
                wc = w2p.tile([P, H], BF, tag=f"wc{e}", name="wc")
                nc.sync.dma_start(wc[:], d["d_wfc2T"][e * P:(e + 1) * P, :])
                wcs.append(wc)
        mu_b, rs_b = ln_stats(xc8, CW)
        xn8 = xnp.tile([P, ET, CW], F8, tag="xn", name="xn")
        ln_apply8(xn8, xc8, mu_b, rs_b, CW)
        if ch == 0:
            q_project(xn8)
        if ch >= 1:
            attention(ch - 1)
        kv_project(xn8, ch)
    attention(NCH - 1)

    close(h_stp, h_ptp, h_sqp, h_xnp, h_xcp, h_kvp, h_wq8p)
    close(h_ps_pv, h_ps_sc, h_ps_mm)

    # ============ proj + residual + LN2 ============
    STAT2, h_STAT2 = pool(name="ps_stat2", bufs=2, space="PSUM")
    ps_mm2, h_ps_mm2 = pool(name="ps_mm2", bufs=4, space="PSUM")
    scratch, _c3 = pool(name="scratch", bufs=2, side="left")
    persistB, h_persistB = pool(name="persistB", bufs=1, side="left")
    x1_sb = persistB.tile([P, ET, TC], F32)
    h2_sb = persistB.tile([P, ET, TC], BF)

    warmup(16, wproj[:, 0, :], ps_mm2)       # bridge attention tail -> proj
    mu2_ps = STAT2.tile([P, TC], F32, tag="mu")
    sq2_ps = STAT2.tile([P, TC], F32, tag="sq")
    for m in range(ET):
        pps = ps_mm2.tile([P, TC], F32, tag="mm", name="pps")
        for e in range(ET):
            nc.tensor.matmul(pps[:], wproj[:, e, m * P:(m + 1) * P],
                             UTs[:, e, :], start=(e == 0), stop=(e == ET - 1))
        # x1 = (proj + bias) + x_slice
        nc.vector.scalar_tensor_tensor(
            x1_sb[:, m, :], pps[:], bproj_sb[:, m:m + 1], xs_sb[:, m, :],
            op0=OP.add, op1=OP.add)
        # LN2 statistics accumulate as each x1 block lands
        xw = scratch.tile([P, TC], BF, tag="ln_xw", name="ln_xw")
        nc.vector.tensor_copy(xw[:], x1_sb[:, m, :])
        x2 = scratch.tile([P, TC], BF, tag="ln_x2", name="ln_x2")
        nc.gpsimd.tensor_mul(x2[:], xw[:], xw[:])
        nc.tensor.matmul(mu2_ps[:], ones_bf[:], xw[:],
                         start=(m == 0), stop=(m == ET - 1), skip_group_check=True)
        nc.tensor.matmul(sq2_ps[:], ones_bf[:], x2[:],
                         start=(m == 0), stop=(m == ET - 1), skip_group_check=True)
    mu_b2, rs_b2 = ln_chain(mu2_ps, sq2_ps, TC)
    for e in range(ET):
        tmp = lnp.tile([P, TC], BF, tag=f"ap{e & 1}", name="tmp2")
        eng = nc.gpsimd if e < 2 else nc.vector
        eng.tensor_sub(tmp[:], x1_sb[:, e, :], mu_b2[:])
        nc.vector.tensor_mul(h2_sb[:, e, :], tmp[:], rs_b2[:])

    # ============ MLP ============
    mlpp, h_mlp = pool(name="mlp", bufs=1, side="left")
    m1_sb = mlpp.tile([P, HT, TC], BF)
    m2_sb = mlpp.tile([P, HT, TC], BF)
    warmup(24, wfc1[:, 0, 0:TC], ps_mm2)     # bridge LN2 chain -> fc1
    for m in range(HT):
        ps1 = ps_mm2.tile([P, TC], F32, tag="mm", name="ps1")
        for e in range(ET):
            nc.tensor.matmul(ps1[:], wfc1[:, e, m * P:(m + 1) * P],
                             h2_sb[:, e, :], start=(e == 0), stop=(e == ET - 1))
        nc.scalar.activation(m1_sb[:, m, :], ps1[:], AF.Relu,
                             bias=bfc1_sb[:, m:m + 1])
    close(h_ps_mm2, h_STAT2)

    # fc2: all 16 weight chunks resident -> one dense 256-matmul run
    ps8p, h_ps8p = pool(name="ps8", bufs=6, space="PSUM")
    for m in range(HT):
        psm = ps8p.tile([P, TC], F32, tag="mm8", name="psm")
        for e in range(HT):
            nc.tensor.matmul(psm[:], wcs[e][:, m * P:(m + 1) * P],
                             m1_sb[:, e, :],
                             start=(e == 0), stop=(e == HT - 1),
                             skip_group_check=True)
        nc.scalar.activation(m2_sb[:, m, :], psm[:], AF.Relu,
                             bias=bfc2_sb[:, m:m + 1])
    close(h_ps8p)

    ps_f3, h_ps_f3 = pool(name="ps_f3", bufs=2, space="PSUM")
    for m in range(ET):
        ps3 = ps_f3.tile([P, TC], F32, tag="f3", name="ps3")
        for e in range(HT):
            nc.tensor.matmul(ps3[:], wfc3[:, e, m * P:(m + 1) * P],
                             m2_sb[:, e, :], start=(e == 0), stop=(e == HT - 1))
        nc.vector.scalar_tensor_tensor(
            x1_sb[:, m, :], ps3[:], bfc3_sb[:, m:m + 1], x1_sb[:, m, :],
            op0=OP.add, op1=OP.add)
        nc.sync.dma_start(d["d_outT"][m * P:(m + 1) * P, :], x1_sb[:, m, :])
    close(h_ps_f3, h_mlp, h_persistB, _c3)
    close(h_w2p, h_w3p, h_w1p, h_wpp, h_persistA, _c1, _c0)


def _get_nc():
    global _BUILT
    if _BUILT is None:
        _BUILT = _build()
    return _BUILT


def run(inputs, trace=False):
    from concourse.bass_utils import run_bass_kernel_spmd
    import ml_dtypes

    nc = _get_nc()
    bf = ml_dtypes.bfloat16
    f8 = ml_dtypes.float8_e4m3fn
    x = np.asarray(inputs["x"], np.float32)[0]          # [T, E]
    g = np.asarray(inputs["ln_g"], np.float32)
    b = np.asarray(inputs["ln_b"], np.float32)

    def fold(w, bias):  # fold LN gamma/beta into consumer weight/bias
        w = np.asarray(w, np.float32)
        return w * g[None, :], np.asarray(bias, np.float32) + w @ b

    qkv_w, bqkv = fold(inputs["qkv_w"], inputs["qkv_b"])
    fc1_w, bfc1 = fold(inputs["fc1_w"], inputs["fc1_b"])

    ct = lambda a: np.ascontiguousarray(np.asarray(a, np.float32).T)
    ctb = lambda a: ct(a).astype(bf)
    to8 = lambda a: np.clip(a, -240.0, 240.0).astype(f8)
    common = {
        "wqkvT8": to8(ct(qkv_w) * WS),
        "bqkv": bqkv,
        "wprojT": ctb(inputs["proj_w"]),
        "bproj": np.asarray(inputs["proj_b"], np.float32),
        "wfc1T": ct(fc1_w).astype(bf),
        "bfc1": bfc1,
        "wfc2T": ctb(inputs["fc2_w"]),
        "bfc2": np.asarray(inputs["fc2_b"], np.float32),
        "wfc3T": ctb(inputs["fc3_w"]),
        "bfc3": np.asarray(inputs["fc3_b"], np.float32),
    }
    in_maps = []
    for c in range(NCORES):
        xrot = np.concatenate([x[c * TC:], x[:c * TC]], axis=0)   # own slice first
        in_maps.append({
            **common,
            "xT8": to8(ct(xrot)),
            "xsT": ct(x[c * TC:(c + 1) * TC, :]),
        })
    res = run_bass_kernel_spmd(nc, in_maps, core_ids=list(range(NCORES)),
                               trace=trace)
    out = np.empty((1, T, E), np.float32)
    for c in range(NCORES):
        out[0, c * TC:(c + 1) * TC, :] = res.results[c]["outT"].T
    return out, res


def kernel(**inputs) -> np.ndarray:
    out, _ = run(inputs, trace=False)
    return out
